# revision 33
# baseline (speedup 1.0000x reference)
"""Trainium2 Bass kernel for nn_DiscreteDecisionEngine.

Math: the reference computes
    q = tanh(geodesic_weights)            # [1, N, 4], N = 256
    h = L(q) (x)  (quaternion Hamilton product per 4-group)
    logits = h_flat @ W.T + b
The Hamilton product is a block-diagonal (4x4 per group) linear map B(q)
applied to x, so logits = x @ (W @ B)^T + b. We fold W' = W @ B on the
host (tiny: [256,1024] weights) and run a pure GEMM on 8 NeuronCores,
data-parallel over the batch.

Device kernel per core (x shard [8192, 1024] f32), DMA-stream-bound:
  for each group of 2 row-tiles (1 MB DMA in, on the SP HWDGE ring):
    per 128-row tile: PE-transpose 128x128 chunks (fp32, 4 per PSUM bank),
    DVE cast-copy -> fp32r (TF32) SBUF, 8 accumulating fp32r matmuls
    psum[128b, 256a] += xT_k.T @ W'T_k, DVE bias-add fused with copyback
    group store [128, 2, 256] via the ACT HWDGE ring
  (software-pipelined one group ahead; last 4 tiles emitted solo to
  shorten the drain)
"""

import os
from contextlib import ExitStack

import numpy as np

import concourse.bass as bass
import concourse.mybir as mybir
import concourse.tile as tile
from concourse import bacc
from concourse.bass import ts
from concourse.bass_utils import run_bass_kernel_spmd
from concourse.masks import make_identity

N_CORES = 8
B_FULL = 65536
B_SHARD = B_FULL // N_CORES  # 8192
D = 1024
A = 256  # num actions
KC = D // 128  # 8 contraction chunks

_F32 = mybir.dt.float32
_F32R = mybir.dt.float32r

# tuning knobs (overridable via env for A/B experiments)
_ACT_COPY_BANK = int(os.environ.get("K_ACT_COPY_BANK", "-1"))
_PIPE = int(os.environ.get("K_PIPE", "1"))
_GROUP = int(os.environ.get("K_GROUP", "2"))  # batch tiles per DMA
_OUT_ON_ACT = bool(int(os.environ.get("K_OUT_ON_ACT", "1")))
_FIRST_SPLIT = int(os.environ.get("K_FIRST_SPLIT", "1024"))  # cols of first sub-load
_TAIL_SPLIT = int(os.environ.get("K_TAIL_SPLIT", "4"))  # trailing tiles emitted solo
_IN_ALT_RING = bool(int(os.environ.get("K_IN_ALT_RING", "0")))
_BUFS_XIN = int(os.environ.get("K_BUFS_XIN", "5"))
_BUFS_TP = int(os.environ.get("K_BUFS_TP", "4"))
_BUFS_XT = int(os.environ.get("K_BUFS_XT", "4"))
_BUFS_PO = int(os.environ.get("K_BUFS_PO", "3"))
_BUFS_OB = int(os.environ.get("K_BUFS_OB", "4"))


def _build_nc():
    nc = bacc.Bacc(None, target_bir_lowering=False)

    x = nc.dram_tensor("x", [B_SHARD, D], _F32, kind="ExternalInput")
    # w[p, k*A + a] = W'[a, 128*k + p]  (host-prepared, SBUF layout)
    w = nc.dram_tensor("w", [128, KC * A], _F32R, kind="ExternalInput")
    # bias broadcast to all 128 partitions on host
    bias = nc.dram_tensor("bias", [128, A], _F32, kind="ExternalInput")
    out = nc.dram_tensor("out", [B_SHARD, A], _F32, kind="ExternalOutput")

    with ExitStack() as ctx:
        tc = ctx.enter_context(tile.TileContext(nc))
        const = ctx.enter_context(tc.tile_pool(name="const", bufs=1))

        xin = ctx.enter_context(tc.tile_pool(name="xin", bufs=_BUFS_XIN))
        tp = ctx.enter_context(tc.tile_pool(name="tp", bufs=_BUFS_TP, space="PSUM"))
        xt = ctx.enter_context(tc.tile_pool(name="xt", bufs=_BUFS_XT))
        po = ctx.enter_context(tc.tile_pool(name="po", bufs=_BUFS_PO, space="PSUM"))
        ob = ctx.enter_context(tc.tile_pool(name="ob", bufs=_BUFS_OB))

        n_tiles = B_SHARD // 128
        G = _GROUP
        # schedule of (first_tile, group_size); optionally split the tail
        # into single-tile groups to shorten the pipeline drain
        tail = min(_TAIL_SPLIT, n_tiles)
        main_tiles = n_tiles - tail
        assert main_tiles % G == 0
        sched = [(i * G, G) for i in range(main_tiles // G)]
        sched += [(main_tiles + j, 1) for j in range(tail)]
        n_groups = len(sched)
        staged = {}

        # first x load is issued before the (1MB) weight load so the PE's
        # transposes start as early as possible; ident is device-generated
        ident = const.tile([128, 128], _F32)
        make_identity(nc, ident)
        xg0 = xin.tile([128, G, D], _F32, tag="xg")
        src0 = x[ts(0, G * 128), :]
        if G > 1:
            src0 = src0.rearrange("(t p) d -> p t d", p=128)
        else:
            src0 = src0.rearrange("p (t d) -> p t d", t=1)
        nc.sync.dma_start(xg0[:, 0, ts(0, _FIRST_SPLIT)], src0[:, 0, ts(0, _FIRST_SPLIT)])
        if _FIRST_SPLIT < D:
            nc.sync.dma_start(
                xg0[:, 0, _FIRST_SPLIT:], src0[:, 0, _FIRST_SPLIT:]
            )
        for t in range(1, G):
            nc.sync.dma_start(xg0[:, t, :], src0[:, t, :])

        # weights/bias ride the ACT HWDGE ring (idle at startup) so they
        # don't delay the x stream on the SP ring
        w_sb = const.tile([128, KC, A], _F32R)
        nc.scalar.dma_start(w_sb[:], w.rearrange("p (k a) -> p k a", k=KC))
        bias_sb = const.tile([128, A], _F32)
        nc.scalar.dma_start(bias_sb[:], bias[:])

        def stage_load_transpose(gi):
            row0, g = sched[gi]
            if gi == 0:
                xg = xg0
            else:
                xg = xin.tile([128, g, D], _F32, tag=f"xg{g}")
                src = x[ts(row0, 128) if g == 1 else bass.ds(row0 * 128, g * 128), :]
                if g > 1:
                    src = src.rearrange("(t p) d -> p t d", p=128)
                else:
                    src = src.rearrange("p (t d) -> p t d", t=1)
                if _IN_ALT_RING and gi % 2 == 1:
                    nc.scalar.dma_start(xg[:], src)
                else:
                    nc.sync.dma_start(xg[:], src)
            xts = []
            for t in range(g):
                xt_tile = xt.tile([128, KC, 128], _F32R, tag="xt")
                for g in range(KC // 4):
                    # 4 transposed chunks per PSUM bank -> single wide copyback
                    pt = tp.tile([128, 4, 128], _F32)
                    for j in range(4):
                        k = 4 * g + j
                        nc.tensor.transpose(
                            pt[:, j, :], xg[:, t, ts(k, 128)], ident[:]
                        )
                    # cast-copy f32 -> f32r (TF32 rounding) for the PE;
                    # optionally alternate banks between DVE and ACT
                    if _ACT_COPY_BANK >= 0 and g % 2 == _ACT_COPY_BANK:
                        nc.scalar.copy(out=xt_tile[:, ts(g, 4), :], in_=pt[:])
                    else:
                        nc.vector.tensor_copy(out=xt_tile[:, ts(g, 4), :], in_=pt[:])
                xts.append(xt_tile)
            staged[gi] = xts

        def stage_matmul_store(gi):
            row0, g = sched[gi]
            xts = staged.pop(gi)
            og = ob.tile([128, g, A], _F32, tag=f"ob{g}")
            for t in range(g):
                p_out = po.tile([128, A], _F32)
                for k in range(KC):
                    nc.tensor.matmul(
                        p_out[:],
                        lhsT=xts[t][:, k, :],
                        rhs=w_sb[:, k, :],
                        start=(k == 0),
                        stop=(k == KC - 1),
                    )
                nc.vector.tensor_add(og[:, t, :], p_out[:], bias_sb[:])
            dst = out[bass.ds(row0 * 128, g * 128), :]
            if g > 1:
                dst = dst.rearrange("(t p) a -> p t a", p=128)
            else:
                dst = dst.rearrange("p (t a) -> p t a", t=1)
            if _OUT_ON_ACT:
                nc.scalar.dma_start(dst, og[:])
            else:
                nc.sync.dma_start(dst, og[:])

        # optional software pipeline: emit transposes of group i+PIPE before
        # matmuls of group i
        for i in range(n_groups + _PIPE):
            if i < n_groups:
                stage_load_transpose(i)
            if i >= _PIPE:
                stage_matmul_store(i - _PIPE)

    nc.finalize()  # runs Bacc.compile(): wait-splitting etc.
    return nc


_NC_CACHE = None
LAST_RESULTS = None


def _get_nc():
    global _NC_CACHE
    if _NC_CACHE is None:
        _NC_CACHE = _build_nc()
    return _NC_CACHE


def _fold_weights(geodesic_weights: np.ndarray, W: np.ndarray) -> np.ndarray:
    """W' = W @ blockdiag(L(tanh(g))^T per 4-group), in float64."""
    q = np.tanh(geodesic_weights.astype(np.float64))[0]  # [N, 4]
    w_, i_, j_, k_ = q[:, 0], q[:, 1], q[:, 2], q[:, 3]
    n = q.shape[0]
    M = np.empty((n, 4, 4), dtype=np.float64)  # y_r = sum_s M[n, r, s] x_s
    M[:, 0] = np.stack([w_, -i_, -j_, -k_], axis=-1)
    M[:, 1] = np.stack([i_, w_, -k_, j_], axis=-1)
    M[:, 2] = np.stack([j_, k_, w_, -i_], axis=-1)
    M[:, 3] = np.stack([k_, -j_, i_, w_], axis=-1)
    W4 = W.astype(np.float64).reshape(A, n, 4)  # [a, n, r]
    Wp = np.einsum("anr,nrs->ans", W4, M).reshape(A, D)
    return Wp.astype(np.float32)  # [a, d]


def kernel(x, geodesic_weights, W, b, **_unused):
    x = np.ascontiguousarray(np.asarray(x, dtype=np.float32))
    Wp = _fold_weights(np.asarray(geodesic_weights), np.asarray(W))
    # device layout: w_dev[p, k*A + a] = Wp[a, 128k + p]
    w_dev = np.ascontiguousarray(
        Wp.T.reshape(KC, 128, A).transpose(1, 0, 2).reshape(128, KC * A)
    )
    bias_dev = np.ascontiguousarray(
        np.broadcast_to(np.asarray(b, dtype=np.float32)[None, :], (128, A))
    )

    nc = _get_nc()
    shards = np.split(x, N_CORES, axis=0)
    in_maps = [{"x": s, "w": w_dev, "bias": bias_dev} for s in shards]
    res = run_bass_kernel_spmd(
        nc,
        in_maps,
        core_ids=list(range(N_CORES)),
        trace=bool(int(os.environ.get("KERNEL_TRACE", "0"))),
    )
    global LAST_RESULTS
    LAST_RESULTS = res
    out = np.concatenate([r["out"] for r in res.results], axis=0)
    return out


# revision 46
# speedup vs baseline: 1.0153x; 1.0153x over previous
"""Trainium2 Bass kernel for nn_DiscreteDecisionEngine.

Math: the reference computes
    q = tanh(geodesic_weights)            # [1, N, 4], N = 256
    h = L(q) (x)  (quaternion Hamilton product per 4-group)
    logits = h_flat @ W.T + b
The Hamilton product is a block-diagonal (4x4 per group) linear map B(q)
applied to x, so logits = x @ (W @ B)^T + b. We fold W' = W @ B on the
host (tiny: [256,1024] weights) and run a pure GEMM on 8 NeuronCores,
data-parallel over the batch.

Device kernel per core (x shard [8192, 1024] f32), DMA-stream-bound:
  for each group of 2 row-tiles (1 MB DMA in, on the SP HWDGE ring):
    per 128-row tile: PE-transpose 128x128 chunks (fp32, 4 per PSUM bank),
    DVE cast-copy -> fp32r (TF32) SBUF, 8 accumulating fp32r matmuls
    psum[128b, 256a] += xT_k.T @ W'T_k, DVE bias-add fused with copyback
    group store [128, 2, 256] via the ACT HWDGE ring
  (software-pipelined one group ahead; last 4 tiles emitted solo to
  shorten the drain)
"""

import os
from contextlib import ExitStack

import numpy as np

import concourse.bass as bass
import concourse.mybir as mybir
import concourse.tile as tile
from concourse import bacc
from concourse.bass import ts
from concourse.bass_utils import run_bass_kernel_spmd
from concourse.masks import make_identity

N_CORES = 8
B_FULL = 65536
B_SHARD = B_FULL // N_CORES  # 8192
D = 1024
A = 256  # num actions
KC = D // 128  # 8 contraction chunks

_F32 = mybir.dt.float32
_F32R = mybir.dt.float32r
_F16 = mybir.dt.float16

# tuning knobs (overridable via env for A/B experiments)
_ACT_COPY_BANK = int(os.environ.get("K_ACT_COPY_BANK", "-1"))
_PIPE = int(os.environ.get("K_PIPE", "1"))
_GROUP = int(os.environ.get("K_GROUP", "2"))  # batch tiles per DMA
_OUT_ON_ACT = bool(int(os.environ.get("K_OUT_ON_ACT", "1")))
_FIRST_SPLIT = int(os.environ.get("K_FIRST_SPLIT", "1024"))  # cols of first sub-load
_TAIL_SPLIT = int(os.environ.get("K_TAIL_SPLIT", "4"))  # trailing tiles emitted solo
_IN_ALT_RING = bool(int(os.environ.get("K_IN_ALT_RING", "0")))
_W_FP16 = bool(int(os.environ.get("K_W_FP16", "1")))  # ship W' as fp16 (exact in TF32)
_TAIL_COLSPLIT = int(os.environ.get("K_TAIL_COLSPLIT", "0"))  # tail groups w/ split loads
_HEAD_SPLIT = int(os.environ.get("K_HEAD_SPLIT", "2"))  # leading tiles emitted solo
_TAIL_ACT = bool(int(os.environ.get("K_TAIL_ACT", "1")))  # ACT copyback in the drain
_BUFS_XIN = int(os.environ.get("K_BUFS_XIN", "5"))
_BUFS_TP = int(os.environ.get("K_BUFS_TP", "4"))
_BUFS_XT = int(os.environ.get("K_BUFS_XT", "4"))
_BUFS_PO = int(os.environ.get("K_BUFS_PO", "3"))
_BUFS_OB = int(os.environ.get("K_BUFS_OB", "4"))


def _build_nc():
    nc = bacc.Bacc(None, target_bir_lowering=False)

    x = nc.dram_tensor("x", [B_SHARD, D], _F32, kind="ExternalInput")
    # w[p, k*A + a] = W'[a, 128*k + p]  (host-prepared, SBUF layout).
    # fp16 halves the transfer; its 11-bit significand matches TF32, so the
    # device-side upconvert to f32r is exact for these magnitudes.
    w = nc.dram_tensor("w", [128, KC * A], _F16 if _W_FP16 else _F32R,
                       kind="ExternalInput")
    # bias broadcast to all 128 partitions on host
    bias = nc.dram_tensor("bias", [128, A], _F32, kind="ExternalInput")
    out = nc.dram_tensor("out", [B_SHARD, A], _F32, kind="ExternalOutput")

    with ExitStack() as ctx:
        tc = ctx.enter_context(tile.TileContext(nc))
        const = ctx.enter_context(tc.tile_pool(name="const", bufs=1))

        xin = ctx.enter_context(tc.tile_pool(name="xin", bufs=_BUFS_XIN))
        tp = ctx.enter_context(tc.tile_pool(name="tp", bufs=_BUFS_TP, space="PSUM"))
        xt = ctx.enter_context(tc.tile_pool(name="xt", bufs=_BUFS_XT))
        po = ctx.enter_context(tc.tile_pool(name="po", bufs=_BUFS_PO, space="PSUM"))
        ob = ctx.enter_context(tc.tile_pool(name="ob", bufs=_BUFS_OB))

        n_tiles = B_SHARD // 128
        G = _GROUP
        # schedule of (first_tile, group_size); head/tail split into
        # single-tile groups to start the PE earlier / shorten the drain
        head = min(_HEAD_SPLIT, n_tiles)
        tail = min(_TAIL_SPLIT, n_tiles - head)
        main_tiles = n_tiles - head - tail
        assert main_tiles % G == 0
        sched = [(j, 1) for j in range(head)]
        sched += [(head + i * G, G) for i in range(main_tiles // G)]
        sched += [(head + main_tiles + j, 1) for j in range(tail)]
        n_groups = len(sched)
        staged = {}

        # first x load is issued before the (1MB) weight load so the PE's
        # transposes start as early as possible; ident is device-generated
        ident = const.tile([128, 128], _F32)
        make_identity(nc, ident)
        g0 = sched[0][1]
        xg0 = xin.tile([128, g0, D], _F32, tag=f"xg{g0}")
        src0 = x[bass.ds(0, g0 * 128), :]
        if g0 > 1:
            src0 = src0.rearrange("(t p) d -> p t d", p=128)
        else:
            src0 = src0.rearrange("p (t d) -> p t d", t=1)
        nc.sync.dma_start(xg0[:, 0, ts(0, _FIRST_SPLIT)], src0[:, 0, ts(0, _FIRST_SPLIT)])
        if _FIRST_SPLIT < D:
            nc.sync.dma_start(
                xg0[:, 0, _FIRST_SPLIT:], src0[:, 0, _FIRST_SPLIT:]
            )
        for t in range(1, g0):
            nc.sync.dma_start(xg0[:, t, :], src0[:, t, :])

        # weights/bias ride the ACT HWDGE ring (idle at startup) so they
        # don't delay the x stream on the SP ring
        w_sb = const.tile([128, KC, A], _F32R)
        if _W_FP16:
            w16 = const.tile([128, KC, A], _F16)
            nc.scalar.dma_start(w16[:], w.rearrange("p (k a) -> p k a", k=KC))
            nc.vector.tensor_copy(out=w_sb[:], in_=w16[:])
        else:
            nc.scalar.dma_start(w_sb[:], w.rearrange("p (k a) -> p k a", k=KC))
        bias_sb = const.tile([128, A], _F32)
        nc.scalar.dma_start(bias_sb[:], bias[:])

        def stage_load_transpose(gi):
            row0, g = sched[gi]
            if gi == 0:
                xg = xg0
            else:
                xg = xin.tile([128, g, D], _F32, tag=f"xg{g}")
                src = x[ts(row0, 128) if g == 1 else bass.ds(row0 * 128, g * 128), :]
                if g > 1:
                    src = src.rearrange("(t p) d -> p t d", p=128)
                else:
                    src = src.rearrange("p (t d) -> p t d", t=1)
                if _IN_ALT_RING and gi % 2 == 1:
                    nc.scalar.dma_start(xg[:], src)
                elif g == 1 and gi >= n_groups - _TAIL_COLSPLIT:
                    # split the last loads by column halves so the drain's
                    # transposes start before the full tile lands
                    nc.sync.dma_start(xg[:, :, : D // 2], src[:, :, : D // 2])
                    nc.sync.dma_start(xg[:, :, D // 2 :], src[:, :, D // 2 :])
                else:
                    nc.sync.dma_start(xg[:], src)
            xts = []
            for t in range(g):
                xt_tile = xt.tile([128, KC, 128], _F32R, tag="xt")
                for g in range(KC // 4):
                    # 4 transposed chunks per PSUM bank -> single wide copyback
                    pt = tp.tile([128, 4, 128], _F32)
                    for j in range(4):
                        k = 4 * g + j
                        nc.tensor.transpose(
                            pt[:, j, :], xg[:, t, ts(k, 128)], ident[:]
                        )
                    # cast-copy f32 -> f32r (TF32 rounding) for the PE;
                    # optionally alternate banks between DVE and ACT
                    in_drain = _TAIL_ACT and row0 >= n_tiles - _TAIL_SPLIT
                    if (_ACT_COPY_BANK >= 0 and g % 2 == _ACT_COPY_BANK) or (
                        in_drain and g % 2 == 1
                    ):
                        nc.scalar.copy(out=xt_tile[:, ts(g, 4), :], in_=pt[:])
                    else:
                        nc.vector.tensor_copy(out=xt_tile[:, ts(g, 4), :], in_=pt[:])
                xts.append(xt_tile)
            staged[gi] = xts

        def stage_matmul_store(gi):
            row0, g = sched[gi]
            xts = staged.pop(gi)
            og = ob.tile([128, g, A], _F32, tag=f"ob{g}")
            for t in range(g):
                p_out = po.tile([128, A], _F32)
                for k in range(KC):
                    nc.tensor.matmul(
                        p_out[:],
                        lhsT=xts[t][:, k, :],
                        rhs=w_sb[:, k, :],
                        start=(k == 0),
                        stop=(k == KC - 1),
                    )
                nc.vector.tensor_add(og[:, t, :], p_out[:], bias_sb[:])
            dst = out[bass.ds(row0 * 128, g * 128), :]
            if g > 1:
                dst = dst.rearrange("(t p) a -> p t a", p=128)
            else:
                dst = dst.rearrange("p (t a) -> p t a", t=1)
            if _OUT_ON_ACT:
                nc.scalar.dma_start(dst, og[:])
            else:
                nc.sync.dma_start(dst, og[:])

        # optional software pipeline: emit transposes of group i+PIPE before
        # matmuls of group i
        for i in range(n_groups + _PIPE):
            if i < n_groups:
                stage_load_transpose(i)
            if i >= _PIPE:
                stage_matmul_store(i - _PIPE)

    nc.finalize()  # runs Bacc.compile(): wait-splitting etc.
    return nc


_NC_CACHE = None
LAST_RESULTS = None


def _get_nc():
    global _NC_CACHE
    if _NC_CACHE is None:
        _NC_CACHE = _build_nc()
    return _NC_CACHE


def _fold_weights(geodesic_weights: np.ndarray, W: np.ndarray) -> np.ndarray:
    """W' = W @ blockdiag(L(tanh(g))^T per 4-group), in float64."""
    q = np.tanh(geodesic_weights.astype(np.float64))[0]  # [N, 4]
    w_, i_, j_, k_ = q[:, 0], q[:, 1], q[:, 2], q[:, 3]
    n = q.shape[0]
    M = np.empty((n, 4, 4), dtype=np.float64)  # y_r = sum_s M[n, r, s] x_s
    M[:, 0] = np.stack([w_, -i_, -j_, -k_], axis=-1)
    M[:, 1] = np.stack([i_, w_, -k_, j_], axis=-1)
    M[:, 2] = np.stack([j_, k_, w_, -i_], axis=-1)
    M[:, 3] = np.stack([k_, -j_, i_, w_], axis=-1)
    W4 = W.astype(np.float64).reshape(A, n, 4)  # [a, n, r]
    Wp = np.einsum("anr,nrs->ans", W4, M).reshape(A, D)
    return Wp.astype(np.float32)  # [a, d]


def kernel(x, geodesic_weights, W, b, **_unused):
    x = np.ascontiguousarray(np.asarray(x, dtype=np.float32))
    Wp = _fold_weights(np.asarray(geodesic_weights), np.asarray(W))
    # device layout: w_dev[p, k*A + a] = Wp[a, 128k + p]
    w_dev = np.ascontiguousarray(
        Wp.T.reshape(KC, 128, A).transpose(1, 0, 2).reshape(128, KC * A)
    )
    if _W_FP16:
        w_dev = w_dev.astype(np.float16)
    bias_dev = np.ascontiguousarray(
        np.broadcast_to(np.asarray(b, dtype=np.float32)[None, :], (128, A))
    )

    nc = _get_nc()
    shards = np.split(x, N_CORES, axis=0)
    in_maps = [{"x": s, "w": w_dev, "bias": bias_dev} for s in shards]
    res = run_bass_kernel_spmd(
        nc,
        in_maps,
        core_ids=list(range(N_CORES)),
        trace=bool(int(os.environ.get("KERNEL_TRACE", "0"))),
    )
    global LAST_RESULTS
    LAST_RESULTS = res
    out = np.concatenate([r["out"] for r in res.results], axis=0)
    return out
